# revision 75
# baseline (speedup 1.0000x reference)
"""Trainium2 Bass kernel for BottleneckAttention (B=32, DIM=512, 4 heads,
dim_head=128, 32x32 spatial, N=1024).

Sharding: data-parallel over batch (4 batches per core x 8 cores).

Per-core pipeline (all matmuls float32r, 1 cycle/col on the PE; the PE is
the bottleneck engine at ~91% occupancy):
  0. dummy ones x ones matmuls at t=0 warm the PE clock gate (HAM) while
     the initial DMAs (single deterministic queue, finest chunks first,
     interleaved with the paired t0/t1 projection consumers) are in flight
  1. qkv projection vs host-transposed weights, emitted in t-pairs so the
     first batch keeps pace with arriving x chunks:
       Q^T,K^T [d,N] channel-major per head (Q pre-scaled by dh^-0.5);
       V [pixel,(h,d)] pixel-major
  2. rel-pos logits via small matmuls whose stationary weights are the
     host-gathered shifted tables RH[xq]/RW[yq] (rel->abs gather folded in)
     -> LHLW^T [64, heads, N]; interleaved with the V projection
  3. S^T[k,q] = K^T.T @ Qs^T (K=128) + E.T @ LHLW^T (K=64, E = one-hot
     expansion) accumulated per (k-chunk, half) in 1-bank PSUM tiles;
     emitted half-pipelined (K0,E0,exp0,K1,E1,exp1) and the projection is
     emitted half-major for the same reason: each half's consumer starts,
     and its PSUM ring slot frees, a half earlier
  4. P^T = exp(S^T) on ScalarE (no max subtraction; logits are O(10));
     all projection PSUM->SBUF copies ride ScalarE so DVE's rel-copy chain
     (which gates the projection-phase PSUM ring) is never delayed
  5. denominator via an off-PE add tree: DVE pair-adds of exp'd chunks,
     gpsimd level-2/3 adds, then a single all-ones [128,128]-stationary
     matmul per half (partition-reduce + broadcast in one, so no gpsimd
     partition_broadcast is needed); the M=1 ones-vector matmuls of the
     original design (17% of PE time) are gone
  6. out^T[d,q] += V_chunk.T @ P chunks, PSUM-accumulated; outp is double-
     buffered in PSUM so the next head's PV never waits on the normalize
  7. each head's denominator matmuls + reciprocals + multiplies + DMA-out
     are deferred into the NEXT head's S phase (pending[0], flushed at kc=5
     -- empirically the best slot) so the PE queue never stalls on the
     DVE/gpsimd tree; both reciprocals are emitted before the multiplies
     because they are what free the dps psA-ring slots.  Batch-boundary and
     final flushes borrow a psO "out"-ring slot for dps instead (the other
     head's buffer is long free there), keeping the psA ring clean for the
     next batch's projection.  The last head feeds tree partials + raw
     pt6/pt7 straight into the matmuls to shorten the final drain.
"""
import numpy as np

import concourse.bass as bass
import concourse.bacc as bacc
import concourse.mybir as mybir
import concourse.tile as tile
from concourse import bass_utils

B, DIM, HEADS, DH, H, W = 32, 512, 4, 128, 32, 32
N = H * W
NCORES = 8
BPC = B // NCORES  # batches per core
SCALE = DH ** -0.5

F32 = mybir.dt.float32
F32R = mybir.dt.float32r
AF = mybir.ActivationFunctionType

_cached_nc = None


def _build_program():
    nc = bacc.Bacc("TRN2", target_bir_lowering=False, debug=False)

    x4 = nc.dram_tensor("x4", [BPC, 128, 4, N], F32, kind="ExternalInput").ap()
    wqk = nc.dram_tensor("wqk", [128, 4, 2 * HEADS * DH], F32, kind="ExternalInput").ap()
    wv = nc.dram_tensor("wv", [128, 4, HEADS * DH], F32, kind="ExternalInput").ap()
    rh = nc.dram_tensor("rh", [128, 32, 32], F32, kind="ExternalInput").ap()
    rw = nc.dram_tensor("rw", [128, 32, 32], F32, kind="ExternalInput").ap()
    em = nc.dram_tensor("em", [64, N], F32, kind="ExternalInput").ap()
    out4 = nc.dram_tensor("out4", [BPC, HEADS * DH, N], F32, kind="ExternalOutput").ap()

    HALF = slice(0, 512), slice(512, 1024)

    with tile.TileContext(nc) as tc:
        with tc.tile_pool(name="cpool", bufs=1) as cpool, \
             tc.tile_pool(name="xpool", bufs=4) as xpool, \
             tc.tile_pool(name="qkpool", bufs=2) as qkpool, \
             tc.tile_pool(name="vpool", bufs=2) as vpool, \
             tc.tile_pool(name="lhlwpool", bufs=1) as lhlwpool, \
             tc.tile_pool(name="ptpool", bufs=4) as ptpool, \
             tc.tile_pool(name="prpool", bufs=3) as prpool, \
             tc.tile_pool(name="sabpool", bufs=1) as sabpool, \
             tc.tile_pool(name="denpool", bufs=1) as denpool, \
             tc.tile_pool(name="outpool", bufs=1) as outpool, \
             tc.tile_pool(name="psA", bufs=4, space="PSUM") as psA, \
             tc.tile_pool(name="psO", bufs=2, space="PSUM") as psO:

            # ---- constants; DMA order matters: the first proj matmul needs
            #      wqk[cc0] + x(b0)[cc0], so those go first --------------------
            e_sb = cpool.tile([64, N], F32R)
            ones_sb = cpool.tile([128, 128], F32R)
            rh_sb = cpool.tile([128, 32, 32], F32R)
            rw_sb = cpool.tile([128, 32, 32], F32R)
            wqk_sb = cpool.tile([128, 4, 2 * HEADS * DH], F32R)
            wv_sb = cpool.tile([128, 4, HEADS * DH], F32R)

            nc.vector.memset(ones_sb.bitcast(F32), 1.0)

            # warm the PE clock gate during the initial DMA window: dummy
            # matmuls (ones x ones) keep the HAM activity monitor busy so the
            # first real matmuls run at full clock (the ramp costs ~3us
            # otherwise); ones_sb doubles as both operands so only its memset
            # gates the first dummy
            warm_ps = psA.tile([128, 512], F32, tag="big", name="warm_ps")
            for _ in range(8):
                nc.tensor.matmul(warm_ps[:, 0:128], ones_sb, ones_sb,
                                 start=True, stop=True)

            prefetched_x = {}
            pending = [None]  # deferred denom+normalize emitter of prev head

            def load_x(bb):
                # batch 0 loads ride the idle ScalarE queue (parallel to the
                # weight DMAs on SP at startup); prefetches for later batches
                # go on SP, whose stream is nearly empty, so the triggers
                # fire as soon as the x slots free up (ScalarE is busy with
                # copies mid-batch and would fire them ~20us late).
                tiles = []
                for cc in range(4):
                    xt = xpool.tile([128, N], F32R, tag="x", name=f"x_{bb}_{cc}")
                    nc.sync.dma_start(out=xt, in_=x4[bb, :, cc, :].bitcast(F32R))
                    tiles.append(xt)
                prefetched_x[bb] = tiles

            # Single deterministic DMA queue for the startup loads, ordered
            # so the paired t0/t1 projection matmuls can chew each x chunk as
            # it lands: finest weight chunk first, then x cc-chunks (both
            # halves) interleaved with the remaining weight chunks, then the
            # tables in first-use order.
            xb0 = []
            for cc in range(4):
                xb0.append(xpool.tile([128, N], F32R, tag="x",
                                      name=f"x_0_{cc}"))
            prefetched_x[0] = xb0
            nc.sync.dma_start(out=wqk_sb[:, 0, 0:256],
                              in_=wqk[:, 0, 0:256].bitcast(F32R))
            nc.sync.dma_start(out=xb0[0][:, 0:512],
                              in_=x4[0, :, 0, 0:512].bitcast(F32R))
            nc.sync.dma_start(out=xb0[0][:, 512:],
                              in_=x4[0, :, 0, 512:].bitcast(F32R))
            nc.sync.dma_start(out=wqk_sb[:, 1:4, 0:256],
                              in_=wqk[:, 1:4, 0:256].bitcast(F32R))
            for cc in range(1, 4):
                nc.sync.dma_start(out=xb0[cc][:, 0:512],
                                  in_=x4[0, :, cc, 0:512].bitcast(F32R))
                nc.sync.dma_start(out=xb0[cc][:, 512:],
                                  in_=x4[0, :, cc, 512:].bitcast(F32R))
            nc.sync.dma_start(out=wqk_sb[:, :, 256:512],
                              in_=wqk[:, :, 256:512].bitcast(F32R))
            nc.sync.dma_start(out=wqk_sb[:, :, 512:768],
                              in_=wqk[:, :, 512:768].bitcast(F32R))
            nc.sync.dma_start(out=rh_sb, in_=rh.bitcast(F32R))
            nc.sync.dma_start(out=wqk_sb[:, :, 768:],
                              in_=wqk[:, :, 768:].bitcast(F32R))
            nc.sync.dma_start(out=rw_sb, in_=rw.bitcast(F32R))
            nc.sync.dma_start(out=wv_sb, in_=wv.bitcast(F32R))
            nc.sync.dma_start(out=e_sb, in_=em.bitcast(F32R))

            for bb in range(BPC):
                x_cc = prefetched_x.pop(bb)

                # ---- qkv projection ---------------------------------------
                # t = kk*HEADS + h; Q tiles (0-3) first so rel can start
                qk_sb = qkpool.tile([128, 8, N], F32R, tag="qk")
                def proj_qk(t):
                    # half-major: pj0's accumulation completes (and its copy
                    # starts) a full half earlier, releasing the psum slot
                    # sooner for the interleaved rel blocks
                    pj0 = psA.tile([128, 512], F32, tag="big", name=f"pj_{t}_0")
                    pj1 = psA.tile([128, 512], F32, tag="big", name=f"pj_{t}_1")
                    cp = nc.scalar.copy
                    for hf, pj in ((0, pj0), (1, pj1)):
                        for cc in range(4):
                            w = wqk_sb[:, cc, t * 128:(t + 1) * 128]
                            nc.tensor.matmul(pj, w, x_cc[cc][:, HALF[hf]],
                                             start=(cc == 0), stop=(cc == 3))
                        cp(qk_sb[:, t, HALF[hf]], pj)
                def proj_qk_pair(ta, tb):
                    # interleave two t-tiles (cc, half)-wise so the first
                    # batch's matmuls keep pace with the arriving x chunks
                    pjs = {t: (psA.tile([128, 512], F32, tag="big",
                                        name=f"pj_{t}_0"),
                               psA.tile([128, 512], F32, tag="big",
                                        name=f"pj_{t}_1"))
                           for t in (ta, tb)}
                    for cc in range(4):
                        for hf in range(2):
                            for t in (ta, tb):
                                w = wqk_sb[:, cc, t * 128:(t + 1) * 128]
                                nc.tensor.matmul(pjs[t][hf], w,
                                                 x_cc[cc][:, HALF[hf]],
                                                 start=(cc == 0),
                                                 stop=(cc == 3))
                    for t in (ta, tb):
                        nc.scalar.copy(qk_sb[:, t, HALF[0]], pjs[t][0])
                        nc.scalar.copy(qk_sb[:, t, HALF[1]], pjs[t][1])

                if bb == 0:
                    # batch 0: cc-outer pair interleave keeps pace with the
                    # arriving x chunks
                    proj_qk_pair(0, 1)
                    proj_qk_pair(2, 3)
                else:
                    for t in range(4):
                        proj_qk(t)
                if pending[0] is not None:
                    # previous batch's last head finishes behind the
                    # projection matmuls of this batch
                    pending[0](True)
                    pending[0] = None
                v_sb = vpool.tile([128, 8, HEADS * DH], F32R, tag="v")
                def proj_v(pc):
                    pv = psA.tile([128, HEADS * DH], F32, tag="big", name=f"pvp_{pc}")
                    for cc in range(4):
                        nc.tensor.matmul(
                            pv,
                            x_cc[cc][:, pc * 128:(pc + 1) * 128],
                            wv_sb[:, cc, :],
                            start=(cc == 0), stop=(cc == 3),
                        )
                    nc.scalar.copy(v_sb[:, pc, :], pv)

                # ---- rel-pos logits ---------------------------------------
                # Pack (2 g-groups) x (4 heads) per matmul: stationary
                # [128, 64] = rh[:, g:g+2, :], moving 256 cols = (g2, t, yq).
                # N=256 keeps fp32r at 1 cyc/row (N<256 runs 4x slower).
                # Valid output: row-half rh <-> g2=rh; junk elsewhere ignored.
                qv4 = qk_sb[:, 0:4, :].rearrange("p t (a b) -> p t a b",
                                                 a=32, b=32)
                lhlw_all = lhlwpool.tile([64, HEADS, N], F32R, tag="lhlw")
                lhv = lhlw_all[0:32].rearrange("p t (a b) -> p t a b", a=32, b=32)
                lwv = lhlw_all[32:64].rearrange("p t (a b) -> p t a b", a=32, b=32)

                def rel_blk(tt, which):
                    tab = rh_sb if which == 0 else rw_sb
                    ps = psA.tile([64, 512], F32, tag="big",
                                  name=f"rel_{which}_{tt}")
                    for mm in range(2):
                        j = tt * 2 + mm
                        g0 = j * 2
                        lhsT = tab[:, g0:g0 + 2, :].rearrange("p a b -> p (a b)")
                        if which == 0:
                            # cols (g2, t, yq): qv4 dims [p, t, g2, yq]
                            rv = qv4[:, :, g0:g0 + 2, :]
                            rhs = bass.AP(tensor=rv.tensor, offset=rv.offset,
                                          ap=[rv.ap[0], rv.ap[2], rv.ap[1], rv.ap[3]])
                        else:
                            # cols (g2, t, xq): qv4 dims [p, t, xq, g2]
                            rv = qv4[:, :, :, g0:g0 + 2]
                            rhs = bass.AP(tensor=rv.tensor, offset=rv.offset,
                                          ap=[rv.ap[0], rv.ap[3], rv.ap[1], rv.ap[2]])
                        nc.tensor.matmul(ps[:, mm * 256:(mm + 1) * 256],
                                         lhsT, rhs, start=True, stop=True)
                    # psum cols: (mm, g2, t, c32); valid g2 == row-half
                    psv = ps.rearrange("p (m g t c) -> p m g t c", m=2, g=2, t=4)
                    for rh_i in range(2):
                        src = psv[rh_i * 32:(rh_i + 1) * 32, :, rh_i, :, :]
                        # src dims [p, mm, t, c]; reorder to [p, t, mm, c]
                        srct = bass.AP(tensor=src.tensor, offset=src.offset,
                                       ap=[src.ap[0], src.ap[2], src.ap[1], src.ap[3]])
                        # g = 4*tt + 2*mm + rh_i
                        if which == 0:
                            # dest xq=g: [p(xk), t, xq {step2}, yq 32]
                            d = lhv[:, :, (4 * tt + rh_i)::2, :]
                            d = bass.AP(tensor=d.tensor, offset=d.offset,
                                        ap=[d.ap[0], d.ap[1],
                                            [d.ap[2][0], 2], d.ap[3]])
                        else:
                            # dest yq=g: [p(yk), t, xq 32, yq {step2}]
                            d0 = lwv[:, :, :, (4 * tt + rh_i)::2]
                            d0 = bass.AP(tensor=d0.tensor, offset=d0.offset,
                                         ap=[d0.ap[0], d0.ap[1], d0.ap[2],
                                             [d0.ap[3][0], 2]])
                            # reorder dest dims to [p, t, yq2, xq]
                            d = bass.AP(tensor=d0.tensor, offset=d0.offset,
                                        ap=[d0.ap[0], d0.ap[1], d0.ap[3], d0.ap[2]])
                        nc.vector.tensor_copy(d, srct)

                # interleave: lh blocks ride proj-qk K-tiles, lw blocks ride
                # proj-v; V psum->sbuf copies go to ScalarE (DVE is the
                # bottleneck of this phase otherwise)
                for t in (4, 5, 6, 7):
                    proj_qk(t)
                    rel_blk(2 * (t - 4), 0)
                    rel_blk(2 * (t - 4) + 1, 0)
                for pc in range(8):
                    proj_v(pc)
                    rel_blk(pc, 1)

                if bb + 1 < BPC:
                    load_x(bb + 1)

                def head_body(bb, h):
                    qs = qk_sb[:, h, :]
                    ks = qk_sb[:, HEADS + h, :]
                    lhlw_sb = lhlw_all[:, h, :]
                    last = (bb == BPC - 1) and (h == HEADS - 1)

                    outp = psO.tile([128, N], F32, tag="out")
                    pt_l = [None] * 8
                    # denominator partial sums (sum of exp'd chunks), built by
                    # a DVE/gpsimd add tree entirely off the PE
                    pr_l = [None] * 4
                    sab_l = [None]
                    sum_l = [None, None]

                    def emit_s(kc):
                        kchunk = slice(kc * 128, (kc + 1) * 128)
                        pt = ptpool.tile([128, N], F32R, tag="pt", name=f"pt_{kc}")
                        pt_l[kc] = pt
                        # K stationary loaded once (both halves), then E
                        # stationary once; exp(half) right after its E-mm
                        s0 = psA.tile([128, 512], F32, tag="big", name=f"sps_{kc}_0")
                        s1 = psA.tile([128, 512], F32, tag="big", name=f"sps_{kc}_1")
                        nc.tensor.matmul(s0, ks[:, kchunk], qs[:, HALF[0]],
                                         start=True, stop=False)
                        nc.tensor.matmul(s0, e_sb[:, kchunk], lhlw_sb[:, HALF[0]],
                                         start=False, stop=True)
                        nc.scalar.activation(out=pt[:, HALF[0]], in_=s0,
                                             func=AF.Exp)
                        nc.tensor.matmul(s1, ks[:, kchunk], qs[:, HALF[1]],
                                         start=True, stop=False)
                        nc.tensor.matmul(s1, e_sb[:, kchunk], lhlw_sb[:, HALF[1]],
                                         start=False, stop=True)
                        nc.scalar.activation(out=pt[:, HALF[1]], in_=s1,
                                             func=AF.Exp)

                    def emit_pair(j):
                        # level-1: pr[j] = pt[2j] + pt[2j+1] on DVE
                        pr = prpool.tile([128, N], F32R, tag="pr",
                                         name=f"pr_{j}")
                        pr_l[j] = pr
                        nc.vector.tensor_add(pr, pt_l[2 * j].bitcast(F32),
                                             pt_l[2 * j + 1].bitcast(F32))

                    def emit_sab():
                        # level-2 for chunks 0-3 only: sab = pr0 + pr1 on
                        # gpsimd (fully off the critical path)
                        sab = sabpool.tile([128, N], F32R, tag="sab")
                        sab_l[0] = sab
                        nc.gpsimd.tensor_add(sab, pr_l[0].bitcast(F32),
                                             pr_l[1].bitcast(F32))

                    def emit_sum1():
                        # level-3a: chunks 0-5, on gpsimd (off the path)
                        s1t = prpool.tile([128, N], F32R, tag="pr",
                                          name="sum1")
                        sum_l[0] = s1t
                        nc.gpsimd.tensor_add(s1t, sab_l[0].bitcast(F32),
                                             pr_l[2].bitcast(F32))

                    def emit_sum2():
                        # level-3b: all chunks, on DVE (short hop after the
                        # last pair add)
                        s2t = prpool.tile([128, N], F32R, tag="pr",
                                          name="sum2")
                        sum_l[1] = s2t
                        nc.vector.tensor_add(s2t, sum_l[0].bitcast(F32),
                                             pr_l[3].bitcast(F32))

                    def emit_pv(kc):
                        pt = pt_l[kc]
                        for half in range(2):
                            nc.tensor.matmul(outp[:, HALF[half]],
                                             v_sb[:, kc, h * DH:(h + 1) * DH],
                                             pt[:, HALF[half]],
                                             start=(kc == 0), stop=(kc == 7))

                    def finish(at_boundary=False):
                        # denominator: dps[p,q] = sum_k P[k,q] via all-ones
                        # [128,128] stationary matmuls (partition reduction +
                        # broadcast to every output partition in one shot).
                        # Mid-batch flushes put the dps halves in psA-ring
                        # slots; batch-boundary flushes borrow the psO "out"
                        # ring instead (the other head's buffer is long free
                        # there), which keeps the psA ring clean for the next
                        # batch's projection.  The last head of the program
                        # skips the sum1/sum2 adds and feeds the tree
                        # partials straight to the matmuls so the final drain
                        # chain is as short as possible.
                        if last:
                            srcs = (sab_l[0], pr_l[2], pt_l[6], pt_l[7])
                        else:
                            srcs = (sum_l[1],)
                        rden = denpool.tile([128, N], F32, tag="rden")
                        out_sb = outpool.tile([128, N], F32, tag="osb")
                        if at_boundary:
                            dpw = psO.tile([128, N], F32, tag="out",
                                           name="dpw")
                            dps = (dpw[:, HALF[0]], dpw[:, HALF[1]])
                        else:
                            dps = tuple(
                                psA.tile([128, 512], F32, tag="big",
                                         name=f"dps_{half}")
                                for half in range(2))
                        for half in range(2):
                            hs = HALF[half]
                            for i, s in enumerate(srcs):
                                nc.tensor.matmul(dps[half], ones_sb, s[:, hs],
                                                 start=(i == 0),
                                                 stop=(i == len(srcs) - 1))
                        # both recips first: they are what frees the dps
                        # ring slots for the next head's S tiles (merged to
                        # one wide op when dps is a single psO tile)
                        if at_boundary:
                            nc.vector.reciprocal(rden, dpw)
                        else:
                            for half in range(2):
                                nc.vector.reciprocal(rden[:, HALF[half]],
                                                     dps[half])
                        for half in range(2):
                            hs = HALF[half]
                            nc.vector.tensor_mul(out_sb[:, hs], outp[:, hs],
                                                 rden[:, hs])
                            nc.sync.dma_start(
                                out=out4[bb, h * DH:(h + 1) * DH, hs],
                                in_=out_sb[:, hs])

                    # PV lags S by 3 kc-iterations: the first pv of a head
                    # must wait until the previous-previous head's PV stream
                    # released the other psO buffer; pt pool holds 4 chunks.
                    for kc in range(8):
                        emit_s(kc)
                        if kc == 3 and pending[0] is not None:
                            # previous head's denom matmuls + normalize ride
                            # behind this head's first S matmuls so the PE
                            # queue never waits on the DVE/gpsimd add tree
                            pending[0]()
                            pending[0] = None
                        if kc >= 3:
                            emit_pv(kc - 3)
                        if kc % 2 == 1 and not (last and kc == 7):
                            emit_pair(kc // 2)
                        if kc == 3:
                            emit_sab()
                        if kc == 5 and not last:
                            emit_sum1()
                    for kc in (6, 7):
                        emit_pv(kc)
                    if not last:
                        emit_sum2()
                    pending[0] = finish

                for h in range(HEADS):
                    head_body(bb, h)

            pending[0](True)
            pending[0] = None

    nc.compile()
    return nc


def _get_program():
    global _cached_nc
    if _cached_nc is None:
        _cached_nc = _build_program()
    return _cached_nc


def _prep_inputs(x, w_qkv, rel_h, rel_w):
    x = np.ascontiguousarray(x, dtype=np.float32)
    w_qkv = np.asarray(w_qkv, dtype=np.float32)
    rel_h = np.asarray(rel_h, dtype=np.float32)
    rel_w = np.asarray(rel_w, dtype=np.float32)

    # x: (B, 512, 32, 32) -> (B, 128, 4, N) with c = cc*128 + p
    x_in = np.ascontiguousarray(
        x.reshape(B, 4, 128, N).transpose(0, 2, 1, 3))

    # w_qkv rows are channels o = d*12 + k*4 + h
    w3 = w_qkv.reshape(DH, 3, HEADS, DIM)         # [d, k, h, c]
    wq_chd = np.transpose(w3[:, 0], (2, 1, 0))    # [c, h, d]
    wk_chd = np.transpose(w3[:, 1], (2, 1, 0))
    wv_chd = np.transpose(w3[:, 2], (2, 1, 0))
    wqk_full = np.concatenate([
        (wq_chd * SCALE).reshape(DIM, HEADS * DH),
        wk_chd.reshape(DIM, HEADS * DH),
    ], axis=1)                                    # [512, 1024]
    wqk_in = np.ascontiguousarray(
        wqk_full.reshape(4, 128, 2 * HEADS * DH).transpose(1, 0, 2))
    wv_in = np.ascontiguousarray(
        wv_chd.reshape(DIM, HEADS * DH).reshape(4, 128, HEADS * DH).transpose(1, 0, 2))

    # shifted rel tables; divide by SCALE because Q is pre-scaled
    idx = np.arange(32)[None, :] - np.arange(32)[:, None] + 31   # [q, k]
    rh_in = np.ascontiguousarray(
        np.transpose(rel_h[idx] / SCALE, (2, 0, 1)))  # [d, xq, xk]
    rw_in = np.ascontiguousarray(
        np.transpose(rel_w[idx] / SCALE, (2, 0, 1)))  # [d, yq, yk]

    # E: [64, N]; rows 0:32 select xk, rows 32:64 select yk
    eye = np.eye(32, dtype=np.float32)
    em_in = np.concatenate([
        np.kron(eye, np.ones((1, 32), dtype=np.float32)),
        np.tile(eye, (1, 32)),
    ], axis=0)

    in_maps = []
    for c in range(NCORES):
        in_maps.append({
            "x4": x_in[c * BPC:(c + 1) * BPC],
            "wqk": wqk_in,
            "wv": wv_in,
            "rh": rh_in,
            "rw": rw_in,
            "em": em_in,
        })
    return in_maps


def run(inputs, trace=False):
    nc = _get_program()
    in_maps = _prep_inputs(**inputs)
    res = bass_utils.run_bass_kernel_spmd(
        nc, in_maps, core_ids=list(range(NCORES)), trace=trace)
    parts = [res.results[c]["out4"].reshape(BPC, HEADS * DH, H, W)
             for c in range(NCORES)]
    out = np.concatenate(parts, axis=0).astype(np.float32)
    return out, res


def kernel(x, w_qkv, rel_h, rel_w):
    out, _ = run(dict(x=x, w_qkv=w_qkv, rel_h=rel_h, rel_w=rel_w))
    return out

